# revision 2
# baseline (speedup 1.0000x reference)
"""ConcatLoRALinear on 8 trn2 NeuronCores, column-parallel over out_features.

Computes out = x @ W.T + b + SCALE * sum_e (x @ A_e.T) @ B_e.T for
x:[4,2048,4096], W:[4096,4096], b:[4096], A:[8,8,4096], B:[8,4096,8].

Strategy (per the column-parallel sharding): each core owns a 512-wide
slice of out_features. x is replicated (pre-transposed on host to [D, T]
so the contraction dim D lands on SBUF partitions). On device, the LoRA
term is folded into the weight first — W_eff.T = W.T + A_cat.T @ (SCALE *
B_cat.T) via rank-64 matmuls — so the main loop is a single dense
[8192x4096] @ [4096x512] matmul in float32r (full PE rate, ~0.5 ulp*2^13
rounding) with the bias added during PSUM eviction.
"""

import numpy as np

import concourse.bass as bass  # noqa: F401  (bass must import before tile)
import concourse.mybir as mybir
import concourse.tile as tile
from concourse import bacc
from concourse.bass_utils import run_bass_kernel_spmd

F32 = mybir.dt.float32
F32R = mybir.dt.float32r

SCALE = 2.0  # alpha/r = 16/8
N_CORES = 8
T = 8192  # tokens = 4*2048
D = 4096  # in_features (contraction)
O_SH = 512  # out_features per core
R_TOT = 64  # E*r = 8*8
KC = 32  # contraction chunks of 128
T_SUPER = 512  # token super-tile (4 PSUM groups of 128)
N_SUPER = T // T_SUPER
KG = 8  # k-chunks per x DMA (2 MiB per dma_start)

_CACHE = {}


def _build():
    nc = bacc.Bacc("TRN2", target_bir_lowering=False, debug=False,
                   num_devices=N_CORES)

    xt_d = nc.dram_tensor("xt", [D, T], F32R, kind="ExternalInput")
    wt_d = nc.dram_tensor("wt", [D, O_SH], F32R, kind="ExternalInput")
    a_d = nc.dram_tensor("acat", [R_TOT, D], F32R, kind="ExternalInput")
    bc_d = nc.dram_tensor("bcat", [R_TOT, O_SH], F32R, kind="ExternalInput")
    bias_d = nc.dram_tensor("bias", [128, O_SH], F32, kind="ExternalInput")
    out_d = nc.dram_tensor("out", [T, O_SH], F32, kind="ExternalOutput")

    # DRAM views with the 128-partition chunk dim split out
    xt_r = xt_d.ap().rearrange("(k p) t -> p k t", p=128)  # [128, KC, T]
    wt_r = wt_d.ap().rearrange("(k p) o -> p k o", p=128)  # [128, KC, O_SH]
    out_r = out_d.ap().rearrange("(t p) o -> p t o", p=128)  # [128, T//128, O_SH]

    with tile.TileContext(nc) as tc:
        with (
            tc.tile_pool(name="const", bufs=1) as const,
            tc.tile_pool(name="wstg_p", bufs=2) as wstg_p,
            tc.tile_pool(name="x_p", bufs=3) as x_p,
            tc.tile_pool(name="o_p", bufs=2) as o_p,
            tc.tile_pool(name="ps_p", bufs=8, space="PSUM") as ps_p,
        ):
            a_sb = const.tile([R_TOT, D], F32R)
            b_sb = const.tile([R_TOT, O_SH], F32R)
            bias_sb = const.tile([128, O_SH], F32)
            weff = const.tile([128, KC * O_SH], F32R)
            nc.sync.dma_start(out=a_sb[:], in_=a_d.ap())
            nc.sync.dma_start(out=b_sb[:], in_=bc_d.ap())
            nc.sync.dma_start(out=bias_sb[:], in_=bias_d.ap())

            # Fold LoRA into the resident weight slice:
            # weff[:, k] = wt[k] + A_cat[:, k-chunk].T @ (SCALE * B_cat.T)
            for kg in range(KC // 4):
                wstg = wstg_p.tile([128, 4, O_SH], F32R, tag="wstg")
                nc.sync.dma_start(
                    out=wstg[:], in_=wt_r[:, kg * 4:(kg + 1) * 4, :]
                )
                for j in range(4):
                    k = kg * 4 + j
                    psf = ps_p.tile([128, O_SH], F32, tag="ps", name="psf")
                    nc.tensor.matmul(
                        psf[:],
                        lhsT=a_sb[:, k * 128:(k + 1) * 128],
                        rhs=b_sb[:],
                        start=True,
                        stop=True,
                    )
                    nc.vector.tensor_tensor(
                        weff[:, k * O_SH:(k + 1) * O_SH],
                        psf[:],
                        wstg[:, j, :],
                        op=mybir.AluOpType.add,
                    )

            # Main loop: out[t, o] = x @ W_eff.T + bias
            for s in range(N_SUPER):
                xs = []
                for g in range(KC // KG):
                    xg = x_p.tile([128, KG, T_SUPER], F32R, tag="xt", name="xg")
                    nc.sync.dma_start(
                        out=xg[:],
                        in_=xt_r[:, g * KG:(g + 1) * KG,
                                 s * T_SUPER:(s + 1) * T_SUPER],
                    )
                    xs.append(xg)
                pss = [
                    ps_p.tile([128, O_SH], F32, tag="ps", name="psacc")
                    for _ in range(T_SUPER // 128)
                ]
                for k in range(KC):
                    xg = xs[k // KG]
                    j = k % KG
                    for ti in range(T_SUPER // 128):
                        nc.tensor.matmul(
                            pss[ti][:],
                            lhsT=xg[:, j, ti * 128:(ti + 1) * 128],
                            rhs=weff[:, k * O_SH:(k + 1) * O_SH],
                            start=(k == 0),
                            stop=(k == KC - 1),
                        )
                ot = o_p.tile([128, T_SUPER // 128, O_SH], F32, tag="ot", name="ot")
                for ti in range(T_SUPER // 128):
                    nc.vector.tensor_tensor(
                        ot[:, ti, :], pss[ti][:], bias_sb[:],
                        op=mybir.AluOpType.add,
                    )
                nc.sync.dma_start(
                    out=out_r[:, s * (T_SUPER // 128):(s + 1) * (T_SUPER // 128), :],
                    in_=ot[:],
                )
    nc.compile()
    return nc


def _shards(x, W, b, A, B):
    """Host-side shard prep. Returns list of 8 in_maps."""
    xt = np.ascontiguousarray(x.reshape(T, D).T)  # [D, T], replicated
    wt = np.ascontiguousarray(W.T)  # [D, O]
    a_cat = np.ascontiguousarray(A.reshape(R_TOT, D))  # row = e*8 + r
    # B_cat.T with scale folded: [R_TOT, O]; row e*8+r matches a_cat
    bc = np.ascontiguousarray(
        (B * SCALE).transpose(0, 2, 1).reshape(R_TOT, D)
    )
    in_maps = []
    for c in range(N_CORES):
        sl = slice(c * O_SH, (c + 1) * O_SH)
        in_maps.append({
            "xt": xt,
            "wt": np.ascontiguousarray(wt[:, sl]),
            "acat": a_cat,
            "bcat": np.ascontiguousarray(bc[:, sl]),
            "bias": np.ascontiguousarray(
                np.broadcast_to(b[sl][None, :], (128, O_SH)).astype(np.float32)
            ),
        })
    return in_maps


def kernel(x, W, b, A, B):
    x = np.asarray(x, dtype=np.float32)
    W = np.asarray(W, dtype=np.float32)
    b = np.asarray(b, dtype=np.float32)
    A = np.asarray(A, dtype=np.float32)
    B = np.asarray(B, dtype=np.float32)

    if "nc" not in _CACHE:
        _CACHE["nc"] = _build()
    nc = _CACHE["nc"]

    in_maps = _shards(x, W, b, A, B)
    res = run_bass_kernel_spmd(nc, in_maps, core_ids=list(range(N_CORES)))
    out = np.concatenate([res.results[c]["out"] for c in range(N_CORES)], axis=1)
    return out.reshape(4, 2048, 4096)
